# revision 24
# baseline (speedup 1.0000x reference)
"""ConsRec-style GNN message passing on 8 Trainium2 NeuronCores — v2.

SpMM strategy: for every 128-edge chunk the host packs (index-select only)
the gathered source rows Xg [128 slots, 64 feats] and a block-diagonal
value matrix Sv [128 slots, M rows]. The device computes
    y^T[:, rows] += Xg^T @ Sv
on the TensorE (both the per-edge scaling and the segment reduction happen
in the PE array); PSUM holds y^T [64, 512 rows] per bank. Rows are sorted
by degree per core and the chunk structure is unified across cores (max of
sorted degree sequences) so one SPMD program serves all 8 cores. All blobs
are bf16; the dense gg matmul runs in fp8 (positive uniform entries, error
averages out over K=10000). Host does data movement only: gathers, sorts,
scatters, dtype casts.
"""
import sys
sys.path.insert(0, '/opt/trn_rl_repo')
import numpy as np
import ml_dtypes

U, I, G = 200000, 100000, 10000
D = 64
M = 8
N_HG = U + G
N_GI = G + I
HGBLK = N_HG // M
P = 128
RT = 512

BF16 = np.dtype(ml_dtypes.bfloat16)
FP8 = np.dtype(ml_dtypes.float8_e4m3)

LAST_EXEC_NS = None
EXEC_NS_PARTS = []


def _rt_pad(n):
    return -(-max(n, 1) // RT) * RT


def _to_u16(x):
    return np.ascontiguousarray(x.astype(BF16).view(np.uint16))


def _build_structure(deg_u):
    """Greedy 128-slot chunks over degree-sorted rows; chunks close at RT-row
    psum-tile boundaries. Shared across cores."""
    Nr = len(deg_u)
    assert Nr % RT == 0
    assert deg_u.max(initial=0) <= P, f"row degree {deg_u.max()} > 128"
    chunk_id = np.zeros(Nr, np.int64)
    slot_base = np.zeros(Nr, np.int64)
    mcol = np.zeros(Nr, np.int64)
    ch_tile, ch_a, ch_M = [], [], []
    q = -1
    for t in range(Nr // RT):
        slots = P + 1
        for i in range(t * RT, (t + 1) * RT):
            d = int(deg_u[i])
            if slots + d > P:
                q += 1
                ch_tile.append(t)
                ch_a.append(i - t * RT)
                ch_M.append(0)
                slots = 0
            chunk_id[i] = q
            slot_base[i] = slots
            mcol[i] = ch_M[q]
            slots += d
            ch_M[q] += 1
    ch_M = np.array(ch_M, np.int64)
    ch_Moff = np.concatenate([[0], np.cumsum(ch_M)[:-1]])
    ch_tile = np.array(ch_tile, np.int64)
    ntile = Nr // RT
    # first chunk index of each tile (chunks are tile-ordered)
    tile_q0 = np.searchsorted(ch_tile, np.arange(ntile + 1))
    return dict(chunk_id=chunk_id, slot_base=slot_base, mcol=mcol,
                ch_tile=ch_tile, ch_a=np.array(ch_a, np.int64),
                ch_M=ch_M, ch_Moff=ch_Moff, nch=q + 1, ntile=ntile,
                tile_q0=tile_q0, Nr=Nr)


def _pack_core(st, sortpos, rows, cols, vals, src_u16):
    """One core's blobs: Xg [128, nch*64] (same int dtype as src), Sv [128, Nr]
    u16(bf16)."""
    nch = st["nch"]
    Mtot = int(st["ch_M"].sum())
    Xg = np.zeros((P, nch, D), src_u16.dtype)
    Sv = np.zeros((P, Mtot), np.uint16)
    if len(rows):
        pos_e = sortpos[rows]
        order = np.argsort(pos_e, kind='stable')
        pos_s = pos_e[order]
        changes = np.flatnonzero(np.diff(pos_s)) + 1
        run_starts = np.concatenate([[0], changes])
        run_lens = np.diff(np.concatenate([run_starts, [len(pos_s)]]))
        rank = np.arange(len(pos_s)) - np.repeat(run_starts, run_lens)
        slot = st["slot_base"][pos_s] + rank
        q = st["chunk_id"][pos_s]
        svcol = st["ch_Moff"][q] + st["mcol"][pos_s]
        Sv[slot, svcol] = _to_u16(vals[order])
        Xg[slot, q, :] = src_u16[cols[order]]
    return Xg.reshape(P, nch * D), Sv


def _sort_unify(rows_by_core, Nr_pad):
    """Degree-sort rows per core; unify chunk structure across cores."""
    degs = [np.bincount(rows_by_core[k], minlength=Nr_pad) for k in range(M)]
    perms = [np.argsort(degs[k], kind='stable') for k in range(M)]
    sortpos = []
    for k in range(M):
        sp = np.empty(Nr_pad, np.int64)
        sp[perms[k]] = np.arange(Nr_pad)
        sortpos.append(sp)
    deg_u = np.max(np.stack([degs[k][perms[k]] for k in range(M)]), axis=0)
    st = _build_structure(deg_u)
    return st, perms, sortpos


def _build_part(rows_by_core, cols_by_core, vals_by_core, Nr_pad, src_u16):
    """Degree-sort rows per core, unify structure, pack per-core blobs.
    Returns (st, perms, sortpos, xg_blobs, sv_blobs)."""
    st, perms, sortpos = _sort_unify(rows_by_core, Nr_pad)
    xgs, svs = [], []
    for k in range(M):
        xg, sv = _pack_core(st, sortpos[k], rows_by_core[k], cols_by_core[k],
                            vals_by_core[k], src_u16)
        xgs.append(xg)
        svs.append(sv)
    return st, perms, sortpos, xgs, svs


def _emit_spmm(nc, mybir, gp, psp, name, xg_d, sv_d, st,
               out_dram=None, out_sb=None, group_tiles=2, psum_tag=None,
               xg_dt=None, out_scale=1.0,
               xg_eng=None, sv_eng=None, out_eng=None):
    """Per psum tile (512 rows): chunks' matmuls into [64, 512] psum bank,
    then one ScalarE copy out. Stages DMA'd group_tiles tiles at a time.
    DMA issue is spread across engine queues to keep SP off the critical
    path."""
    f32 = mybir.dt.float32
    if xg_dt is None:
        xg_dt = mybir.dt.bfloat16
    xg_eng = xg_eng or nc.sync
    sv_eng = sv_eng or nc.sync
    out_eng = out_eng or nc.sync
    ntile = st["ntile"]
    tq = st["tile_q0"]
    Moffs = st["ch_Moff"]
    Ms = st["ch_M"]
    As = st["ch_a"]
    for g0 in range(0, ntile, group_tiles):
        g1 = min(g0 + group_tiles, ntile)
        q0, q1 = int(tq[g0]), int(tq[g1])
        if q1 == q0:
            continue
        m0 = int(Moffs[q0])
        m1 = int(Moffs[q1 - 1] + Ms[q1 - 1])
        xg_t = gp.tile([P, (q1 - q0) * D], xg_dt, tag=f"xg_{name}")
        xg_eng.dma_start(xg_t[:], xg_d[:, q0 * D:q1 * D])
        sv_t = gp.tile([P, m1 - m0], mybir.dt.bfloat16, tag=f"sv_{name}")
        sv_eng.dma_start(sv_t[:], sv_d[:, m0:m1])
        if out_dram is not None:
            o_t = gp.tile([D, (g1 - g0) * RT], mybir.dt.bfloat16,
                          tag=f"o_{name}")
        for t in range(g0, g1):
            ps = psp.tile([D, RT], f32, tag=psum_tag or f"ps_{name}")
            for q in range(int(tq[t]), int(tq[t + 1])):
                a = int(As[q]); Mq = int(Ms[q]); Mo = int(Moffs[q])
                nc.tensor.matmul(
                    ps[:, a:a + Mq],
                    xg_t[:, (q - q0) * D:(q - q0 + 1) * D],
                    sv_t[:, Mo - m0:Mo - m0 + Mq],
                    start=True, stop=True)
            if out_dram is not None:
                nc.scalar.activation(o_t[:, (t - g0) * RT:(t - g0 + 1) * RT],
                                     ps[:], mybir.ActivationFunctionType.Copy,
                                     scale=out_scale)
            else:
                nc.scalar.activation(out_sb[:, t * RT:(t + 1) * RT], ps[:],
                                     mybir.ActivationFunctionType.Copy,
                                     scale=out_scale)
        if out_dram is not None:
            out_eng.dma_start(out_dram[:, g0 * RT:g1 * RT], o_t[:])


def _run(nc, in_maps, label):
    import os
    from concourse.bass_utils import run_bass_kernel_spmd
    trace = bool(os.environ.get("BASS_TRACE"))
    try:
        import antenv.axon_hooks  # noqa: F401
    except ModuleNotFoundError:
        os.environ["BASS_NEVER_TRACE"] = "1"
        trace = False
    if os.environ.get("BASS_SIM"):
        from concourse.timeline_sim import TimelineSim
        ts = TimelineSim(nc, trace=bool(os.environ.get("BASS_SIM_TRACE")))
        t = ts.simulate()
        EXEC_NS_PARTS.append((label + "(sim)", int(t)))
        print(f"  [sim] launch {label}: {t:.0f}")
        if ts.perfetto is not None:
            ts.perfetto.save(f"/tmp/sim_{label}.pftrace")
        if os.environ.get("BASS_SIM_ONLY"):
            from concourse import mybir as _mb
            outs = {}
            for alloc in nc.m.functions[0].allocations:
                if getattr(alloc, "kind", None) == "ExternalOutput":
                    outs[alloc.memorylocations[0].name] = np.zeros(
                        tuple(alloc.tensor_shape), _mb.dt.np(alloc.dtype))
            return [dict(outs) for _ in range(M)]
    res = run_bass_kernel_spmd(nc, in_maps, list(range(M)), trace=trace)
    if res.exec_time_ns is not None:
        EXEC_NS_PARTS.append((label, res.exec_time_ns))
    return res.results


def _split_pad(arr, width):
    out = np.full((M, width), -1, np.int64)
    for k, p in enumerate(np.array_split(arr, M)):
        out[k, :len(p)] = p
    return out


def kernel(user_inputs, pos_groups, neg_groups,
           hg_rows, hg_cols, hg_vals,
           gi_rows, gi_cols, gi_vals,
           gg_graph,
           user_emb, item_emb, group_emb,
           hyper_w, hyper_b, lightgcn_w, lightgcn_b, overlap_w, overlap_b):
    global LAST_EXEC_NS, EXEC_NS_PARTS
    EXEC_NS_PARTS = []
    import concourse.bacc as bacc
    import concourse.tile as tile
    from concourse import mybir
    f32 = mybir.dt.float32
    bf16 = mybir.dt.bfloat16
    fp8 = mybir.dt.float8e4

    user_inputs = np.asarray(user_inputs).astype(np.int64)
    pos_groups = np.asarray(pos_groups).astype(np.int64)
    neg_groups = np.asarray(neg_groups).astype(np.int64)
    hg_rows = np.asarray(hg_rows).astype(np.int64)
    hg_cols = np.asarray(hg_cols).astype(np.int64)
    hg_vals = np.asarray(hg_vals).astype(np.float32)
    gi_rows = np.asarray(gi_rows).astype(np.int64)
    gi_cols = np.asarray(gi_cols).astype(np.int64)
    gi_vals = np.asarray(gi_vals).astype(np.float32)
    gg_graph = np.asarray(gg_graph).astype(np.float32)
    user_emb = np.asarray(user_emb).astype(np.float32)
    item_emb = np.asarray(item_emb).astype(np.float32)
    group_emb = np.asarray(group_emb).astype(np.float32)

    x0_u16 = np.concatenate([_to_u16(user_emb), _to_u16(group_emb)], axis=0)
    xgi_u16 = np.concatenate([_to_u16(group_emb), _to_u16(item_emb)], axis=0)
    # fp8 (x32 scaled) copies of the L1/L2 gather sources; device de-scales
    # by 1/32 in the psum->sbuf copy
    XS = np.float32(32.0)
    x0_u8 = np.ascontiguousarray(
        (np.concatenate([user_emb, group_emb], axis=0) * XS)
        .astype(FP8).view(np.uint8))

    selU = np.unique(user_inputs)
    selG = np.unique(np.concatenate([pos_groups, neg_groups]))
    NPU = _rt_pad(-(-len(selU) // M))
    NPG = _rt_pad(-(-len(selG) // M))
    selU_sh = _split_pad(selU, NPU)
    selG_sh = _split_pad(selG, NPG)

    # row -> (core, local) maps for psel rows (hg node ids)
    rm_core = np.full(N_HG, -1, np.int32)
    rm_loc = np.full(N_HG, -1, np.int32)
    for k in range(M):
        vu = selU_sh[k] >= 0
        rm_core[selU_sh[k][vu]] = k
        rm_loc[selU_sh[k][vu]] = np.nonzero(vu)[0]
        vg = selG_sh[k] >= 0
        rm_core[U + selG_sh[k][vg]] = k
        rm_loc[U + selG_sh[k][vg]] = np.nonzero(vg)[0]

    # ---- L3 edges (rows in psel), split into user part and group part ----
    m3 = rm_core[hg_rows] >= 0
    r3 = hg_rows[m3]
    c3 = hg_cols[m3]
    v3 = hg_vals[m3]
    isu3 = r3 < U
    e3u = (rm_core[r3[isu3]], rm_loc[r3[isu3]].astype(np.int64),
           c3[isu3], v3[isu3])
    e3g = (rm_core[r3[~isu3]], rm_loc[r3[~isu3]].astype(np.int64),
           c3[~isu3], v3[~isu3])

    # ---- L2 rows: cols of L3 edges + psel itself ----
    psel = np.concatenate([selU, U + selG])
    needed2 = np.unique(np.concatenate([c3, psel]))
    n2len = len(needed2)
    R2 = _rt_pad(-(-n2len // M))
    bounds = [len(p) for p in np.array_split(needed2, M)]
    off2 = np.concatenate([[0], np.cumsum(bounds)])
    n2_core = np.full(N_HG, -1, np.int32)
    n2_loc = np.full(N_HG, -1, np.int64)
    for k in range(M):
        lo, hi = off2[k], off2[k + 1]
        rows_k = needed2[lo:hi]
        n2_core[rows_k] = k
        n2_loc[rows_k] = np.arange(hi - lo)

    m2 = n2_core[hg_rows] >= 0
    e2 = (n2_core[hg_rows[m2]], n2_loc[hg_rows[m2]],
          hg_cols[m2], hg_vals[m2])

    # ---- L1: all hg edges, block row shard ----
    e1_core = (hg_rows // HGBLK).astype(np.int32)
    e1_loc = hg_rows % HGBLK
    NR1 = _rt_pad(HGBLK)

    # ---- gi edges: rows in selG ----
    gmask = gi_rows < G
    grow = gi_rows[gmask]
    gc = rm_core[U + grow]
    glc = rm_loc[U + grow].astype(np.int64)
    mg = gc >= 0
    egi = (gc[mg], glc[mg], gi_cols[gmask][mg], gi_vals[gmask][mg])

    def by_core(e):
        cores, locs, cols, vals = e
        out = []
        for k in range(M):
            m = cores == k
            out.append((locs[m], cols[m], vals[m]))
        return out

    # ---- L3 structures early: launch A computes gg in L3-groups perm order
    e3u_by = by_core(e3u)
    e3g_by = by_core(e3g)
    st3u, perm3u, spos3u = _sort_unify([e[0] for e in e3u_by], NPU)
    st3g, perm3g, spos3g = _sort_unify([e[0] for e in e3g_by], NPG)

    GG_SCALE = 4096.0
    KCH = -(-G // P)
    GPAD = KCH * P
    gemb_pad = np.zeros((GPAD, D), np.float32)
    gemb_pad[:G] = group_emb
    gemb8 = np.ascontiguousarray(
        gemb_pad.reshape(KCH, P, D).transpose(1, 0, 2).reshape(P, KCH * D)
    ).astype(FP8)
    ggpm_by = []
    for k in range(M):
        gsel = selG_sh[k][perm3g[k]]
        ggr = np.zeros((NPG, G), np.float32)
        vmask = gsel >= 0
        ggr[np.nonzero(vmask)[0]] = gg_graph[gsel[vmask]]
        ggp = np.zeros((GPAD, NPG), np.float32)
        ggp[:G] = ggr.T
        # fp8 e4m3 subnormals start at 2^-9; gg entries are ~1e-4, so fold a
        # 2^12 scale into the quantization (device de-scales in the psum copy)
        ggpm_by.append(np.ascontiguousarray(
            ggp.reshape(KCH, P, NPG).transpose(1, 0, 2).reshape(P, KCH * NPG)
            * np.float32(GG_SCALE)).astype(FP8))

    # ================= launch A: L1 + gi + gg =================
    e1_by = [(e1_loc[e1_core == k], hg_cols[e1_core == k],
              hg_vals[e1_core == k]) for k in range(M)]
    st1, perm1, spos1, xg1, sv1 = _build_part(
        [e[0] for e in e1_by], [e[1] for e in e1_by], [e[2] for e in e1_by],
        NR1, x0_u8)
    egi_by = by_core(egi)
    stg, permg, sposg, xgg, svg = _build_part(
        [e[0] for e in egi_by], [e[1] for e in egi_by], [e[2] for e in egi_by],
        NPG, xgi_u16)

    NB = NPG // RT
    ncA = bacc.Bacc(None, target_bir_lowering=False, debug=False)
    xg1_d = ncA.dram_tensor("xg1", [P, st1["nch"] * D], fp8, kind="ExternalInput")
    sv1_d = ncA.dram_tensor("sv1", [P, NR1], bf16, kind="ExternalInput")
    y1_d = ncA.dram_tensor("y1", [D, NR1], bf16, kind="ExternalOutput")
    xgg_d = ncA.dram_tensor("xgg", [P, stg["nch"] * D], bf16, kind="ExternalInput")
    svg_d = ncA.dram_tensor("svg", [P, NPG], bf16, kind="ExternalInput")
    yg_d = ncA.dram_tensor("yg", [D, NPG], bf16, kind="ExternalOutput")
    ggpm_d = ncA.dram_tensor("ggpm", [P, KCH * NPG], fp8, kind="ExternalInput")
    gemb_d = ncA.dram_tensor("gemb8", [P, KCH * D], fp8, kind="ExternalInput")
    b3_d = ncA.dram_tensor("b3g", [D, NPG], bf16, kind="ExternalOutput")
    with tile.TileContext(ncA) as tc:
        with (
            tc.tile_pool(name="gp", bufs=3) as gp,
            tc.tile_pool(name="fus", bufs=1) as fus,
            tc.tile_pool(name="ps", bufs=3, space="PSUM") as psp,
            tc.tile_pool(name="psg", bufs=1, space="PSUM") as psg,
        ):
            # gg dense matmul (fp8 DoubleRow), overlapped with the spmm DMAs
            gemb_t = fus.tile([P, KCH, D], fp8, tag="gemb")
            ncA.sync.dma_start(gemb_t[:], gemb_d[:].rearrange(
                "p (c d) -> p c d", c=KCH))
            ps_gg = [psg.tile([D, RT], f32, tag=f"psgg{b}", name=f"psgg{b}")
                     for b in range(NB)]
            GGST = 8
            for c0 in range(0, KCH, GGST):
                c1 = min(c0 + GGST, KCH)
                gg_t = gp.tile([P, c1 - c0, NPG], fp8, tag="ggst")
                ncA.gpsimd.dma_start(gg_t[:], ggpm_d[:, c0 * NPG:c1 * NPG]
                                     .rearrange("p (c n) -> p c n", c=c1 - c0))
                c = c0
                while c < c1:
                    pair = 2 if c + 1 < c1 else 1
                    for b in range(NB):
                        ncA.tensor.matmul(
                            ps_gg[b][:],
                            gemb_t[:, c:c + pair, :],
                            gg_t[:, c - c0:c - c0 + pair,
                                 b * RT:(b + 1) * RT],
                            start=(c == 0), stop=(c + pair == KCH),
                            perf_mode=(mybir.MatmulPerfMode.DoubleRow
                                       if pair == 2 else None))
                    c += pair
            b3o_t = fus.tile([D, NPG], bf16, tag="b3o")
            for b in range(NB):
                ncA.scalar.activation(b3o_t[:, b * RT:(b + 1) * RT],
                                      ps_gg[b][:],
                                      mybir.ActivationFunctionType.Copy,
                                      scale=1.0 / GG_SCALE)
            ncA.scalar.dma_start(b3_d[:], b3o_t[:])

            _emit_spmm(ncA, mybir, gp, psp, "l1", xg1_d, sv1_d, st1,
                       out_dram=y1_d, group_tiles=4, xg_dt=fp8,
                       out_scale=1.0 / 32.0,
                       sv_eng=ncA.gpsimd, out_eng=ncA.scalar)
            _emit_spmm(ncA, mybir, gp, psp, "gi", xgg_d, svg_d, stg,
                       out_dram=yg_d, group_tiles=2,
                       sv_eng=ncA.gpsimd, out_eng=ncA.scalar)
    ncA.compile()
    mapsA = [{"xg1": xg1[k].view(FP8), "sv1": sv1[k].view(BF16),
              "xgg": xgg[k].view(BF16), "svg": svg[k].view(BF16),
              "ggpm": ggpm_by[k], "gemb8": gemb8}
             for k in range(M)]
    resA = _run(ncA, mapsA, "A")

    # assemble y1 table (bf16-as-u16), gi rows, gg branch result
    y1_u16 = np.zeros((N_HG, D), np.uint16)
    gi_loc_u16 = []
    b3_by = []
    for k in range(M):
        yt = np.asarray(resA[k]["y1"]).view(np.uint16)  # [64, NR1]
        y_sorted = np.ascontiguousarray(yt.T)
        y1_u16[k * HGBLK:(k + 1) * HGBLK] = y_sorted[spos1[k][:HGBLK]]
        gt = np.asarray(resA[k]["yg"]).view(np.uint16)
        gi_loc_u16.append(np.ascontiguousarray(gt.T)[sposg[k]])
        b3_by.append(np.asarray(resA[k]["b3g"]))        # [64, NPG] bf16

    # ================= launch B: L2 =================
    y1_u8 = np.ascontiguousarray(
        (y1_u16.view(BF16).astype(np.float32) * XS).astype(FP8).view(np.uint8))
    e2_by = by_core(e2)
    st2, perm2, spos2, xg2, sv2 = _build_part(
        [e[0] for e in e2_by], [e[1] for e in e2_by], [e[2] for e in e2_by],
        R2, y1_u8)
    ncB = bacc.Bacc(None, target_bir_lowering=False, debug=False)
    xg2_d = ncB.dram_tensor("xg2", [P, st2["nch"] * D], fp8, kind="ExternalInput")
    sv2_d = ncB.dram_tensor("sv2", [P, R2], bf16, kind="ExternalInput")
    y2_d = ncB.dram_tensor("y2", [D, R2], bf16, kind="ExternalOutput")
    with tile.TileContext(ncB) as tc:
        with (
            tc.tile_pool(name="gp", bufs=3) as gp,
            tc.tile_pool(name="ps", bufs=4, space="PSUM") as psp,
        ):
            _emit_spmm(ncB, mybir, gp, psp, "l2", xg2_d, sv2_d, st2,
                       out_dram=y2_d, group_tiles=4, xg_dt=fp8,
                       out_scale=1.0 / 32.0,
                       sv_eng=ncB.gpsimd, out_eng=ncB.scalar)
    ncB.compile()
    mapsB = [{"xg2": xg2[k].view(FP8), "sv2": sv2[k].view(BF16)}
             for k in range(M)]
    resB = _run(ncB, mapsB, "B")

    y2_u16 = np.zeros((N_HG, D), np.uint16)
    for k in range(M):
        yt = np.asarray(resB[k]["y2"]).view(np.uint16)
        y_sorted = np.ascontiguousarray(yt.T)
        lo, hi = off2[k], off2[k + 1]
        y2_u16[needed2[lo:hi]] = y_sorted[spos2[k][:hi - lo]]

    # ================= launch C: L3 + fusion =================
    xg3u, sv3u, xg3g, sv3g = [], [], [], []
    for k in range(M):
        xg, sv = _pack_core(st3u, spos3u[k], *e3u_by[k], y2_u16)
        xg3u.append(xg)
        sv3u.append(sv)
        xg, sv = _pack_core(st3g, spos3g[k], *e3g_by[k], y2_u16)
        xg3g.append(xg)
        sv3g.append(sv)

    # h-vectors in part-sorted order, transposed [64, Nr] bf16
    def hvecs(sel_rows, perm, table_u16):
        rows = np.where(sel_rows >= 0, sel_rows, 0)[perm]
        hv = table_u16[rows]            # [Nr, 64]
        return np.ascontiguousarray(hv.T)

    hostmaps = []
    for k in range(M):
        rows_u = np.where(selU_sh[k] >= 0, selU_sh[k], 0)
        rows_g = np.where(selG_sh[k] >= 0, U + selG_sh[k], 0)
        hm = {
            "xg3u": xg3u[k].view(BF16), "sv3u": sv3u[k].view(BF16),
            "xg3g": xg3g[k].view(BF16), "sv3g": sv3g[k].view(BF16),
            "hxu": hvecs(rows_u, perm3u[k], x0_u16).view(BF16),
            "h1u": hvecs(rows_u, perm3u[k], y1_u16).view(BF16),
            "h2u": hvecs(rows_u, perm3u[k], y2_u16).view(BF16),
            "hxg": hvecs(rows_g, perm3g[k], x0_u16).view(BF16),
            "h1g": hvecs(rows_g, perm3g[k], y1_u16).view(BF16),
            "h2g": hvecs(rows_g, perm3g[k], y2_u16).view(BF16),
            "gio": np.ascontiguousarray(
                gi_loc_u16[k][perm3g[k]].T).view(BF16),
            "b3g": b3_by[k],
            "w_h": hyper_w.astype(np.float32).reshape(D, 1),
            "w_l": lightgcn_w.astype(np.float32).reshape(D, 1),
            "w_o": overlap_w.astype(np.float32).reshape(D, 1),
        }
        hostmaps.append(hm)

    b_h = float(np.asarray(hyper_b).reshape(-1)[0])
    b_l = float(np.asarray(lightgcn_b).reshape(-1)[0])
    b_o = float(np.asarray(overlap_b).reshape(-1)[0])

    ncC = bacc.Bacc(None, target_bir_lowering=False, debug=False)
    dr = {}
    dr["xg3u"] = ncC.dram_tensor("xg3u", [P, st3u["nch"] * D], bf16, kind="ExternalInput")
    dr["sv3u"] = ncC.dram_tensor("sv3u", [P, NPU], bf16, kind="ExternalInput")
    dr["xg3g"] = ncC.dram_tensor("xg3g", [P, st3g["nch"] * D], bf16, kind="ExternalInput")
    dr["sv3g"] = ncC.dram_tensor("sv3g", [P, NPG], bf16, kind="ExternalInput")
    for n in ("hxu", "h1u", "h2u"):
        dr[n] = ncC.dram_tensor(n, [D, NPU], bf16, kind="ExternalInput")
    for n in ("hxg", "h1g", "h2g", "gio"):
        dr[n] = ncC.dram_tensor(n, [D, NPG], bf16, kind="ExternalInput")
    dr["b3g"] = ncC.dram_tensor("b3g", [D, NPG], bf16, kind="ExternalInput")
    for n in ("w_h", "w_l", "w_o"):
        dr[n] = ncC.dram_tensor(n, [D, 1], f32, kind="ExternalInput")
    users_d = ncC.dram_tensor("users_out", [D, NPU], f32, kind="ExternalOutput")
    groups_d = ncC.dram_tensor("groups_out", [D, NPG], f32, kind="ExternalOutput")

    with tile.TileContext(ncC) as tc:
        with (
            tc.tile_pool(name="gp", bufs=2) as gp,
            tc.tile_pool(name="fus", bufs=1) as fus,
            tc.tile_pool(name="tmp", bufs=2) as tmp,
            tc.tile_pool(name="ps", bufs=2, space="PSUM") as psp,
            tc.tile_pool(name="psd", bufs=2, space="PSUM") as psd,
        ):
            # ---- gg branch result computed in launch A; load + widen ----
            b3in_t = fus.tile([D, NPG], mybir.dt.bfloat16, tag="b3in")
            ncC.scalar.dma_start(b3in_t[:], dr["b3g"][:])
            b3_t = fus.tile([D, NPG], f32, tag="b3")
            ncC.scalar.activation(b3_t[:], b3in_t[:],
                                  mybir.ActivationFunctionType.Copy)

            # ---- L3 spmm parts, outputs resident in SBUF (f32) ----
            y3u_t = fus.tile([D, NPU], f32, tag="y3u")
            _emit_spmm(ncC, mybir, gp, psp, "l3", dr["xg3u"], dr["sv3u"],
                       st3u, out_sb=y3u_t, group_tiles=1, psum_tag="ps_l3",
                       sv_eng=ncC.gpsimd)
            y3g_t = fus.tile([D, NPG], f32, tag="y3g")
            _emit_spmm(ncC, mybir, gp, psp, "l3", dr["xg3g"], dr["sv3g"],
                       st3g, out_sb=y3g_t, group_tiles=1, psum_tag="ps_l3",
                       sv_eng=ncC.gpsimd)

            # ---- h vectors ----
            h_t = {}
            for n in ("hxu", "h1u", "h2u", "hxg", "h1g", "h2g", "gio"):
                wdt = NPU if n.endswith("u") else NPG
                h_t[n] = fus.tile([D, wdt], bf16, tag=n, name=f"h_{n}")
                ncC.scalar.dma_start(h_t[n][:], dr[n][:])

            # ---- acc = 0.25*(hx+h1+h2+y3) for users and groups ----
            def emit_acc(hx, h1, h2, y3, out_tile):
                ta = tmp.tile([D, NPU], f32, tag="accA", name="accA")
                ncC.vector.tensor_add(ta[:], hx[:], h1[:])
                tb = tmp.tile([D, NPU], f32, tag="accB", name="accB")
                ncC.vector.tensor_add(tb[:], h2[:], y3[:])
                tcs = tmp.tile([D, NPU], f32, tag="accC", name="accC")
                ncC.vector.tensor_add(tcs[:], ta[:], tb[:])
                ncC.vector.tensor_scalar_mul(out_tile[:], tcs[:], 0.25)

            accu = fus.tile([D, NPU], f32, tag="accu")
            emit_acc(h_t["hxu"], h_t["h1u"], h_t["h2u"], y3u_t, accu)
            ncC.sync.dma_start(users_d[:], accu[:])

            b1_t = fus.tile([D, NPG], f32, tag="b1")
            emit_acc(h_t["hxg"], h_t["h1g"], h_t["h2g"], y3g_t, b1_t)

            b2_t = fus.tile([D, NPG], f32, tag="b2")
            ncC.scalar.activation(b2_t[:], h_t["gio"][:],
                                  mybir.ActivationFunctionType.Copy)

            # ---- gating ----
            w_t = {}
            for n in ("w_h", "w_l", "w_o"):
                w_t[n] = fus.tile([D, 1], f32, tag=n, name=f"wt_{n}")
                ncC.sync.dma_start(w_t[n][:], dr[n][:])
            ones_t = fus.tile([1, D], f32, tag="ones")
            ncC.vector.memset(ones_t[:], 1.0)

            outs = [fus.tile([D, NPG], f32, tag="outg0", name="outg0"),
                    fus.tile([D, NPG], f32, tag="outg1", name="outg1")]
            out_t = None
            for bi, (n, br, bias) in enumerate(
                    (("w_h", b1_t, b_h), ("w_l", b2_t, b_l),
                     ("w_o", b3_t, b_o))):
                coef = tmp.tile([1, NPG], f32, tag="coef", name="coef")
                for b in range(NB):
                    ps_dot = psd.tile([1, RT], f32, tag="dot", name="psdot")
                    ncC.tensor.matmul(ps_dot[:], w_t[n][:],
                                      br[:, b * RT:(b + 1) * RT],
                                      start=True, stop=True)
                    ncC.scalar.activation(coef[:, b * RT:(b + 1) * RT],
                                          ps_dot[:],
                                          mybir.ActivationFunctionType.Sigmoid,
                                          bias=bias)
                contrib = tmp.tile([D, NPG], f32, tag="ctr", name="ctr")
                for b in range(NB):
                    ps_rep = psd.tile([D, RT], f32, tag="rep", name="psrep")
                    ncC.tensor.matmul(ps_rep[:], ones_t[:],
                                      coef[:, b * RT:(b + 1) * RT],
                                      start=True, stop=True)
                    ncC.vector.tensor_mul(contrib[:, b * RT:(b + 1) * RT],
                                          br[:, b * RT:(b + 1) * RT],
                                          ps_rep[:])
                if bi == 0:
                    ncC.vector.tensor_copy(outs[0][:], contrib[:])
                    out_t = outs[0]
                else:
                    nxt = outs[bi % 2]
                    ncC.vector.tensor_add(nxt[:], out_t[:], contrib[:])
                    out_t = nxt
            ncC.sync.dma_start(groups_d[:], out_t[:])
    ncC.compile()
    resC = _run(ncC, hostmaps, "C")

    if EXEC_NS_PARTS:
        real = [t for lbl, t in EXEC_NS_PARTS if not lbl.endswith("(sim)")]
        sims = [t for lbl, t in EXEC_NS_PARTS if lbl.endswith("(sim)")]
        LAST_EXEC_NS = int(sum(real)) if real else int(sum(sims))

    users_tab = np.zeros((U, D), np.float32)
    groups_tab = np.zeros((G, D), np.float32)
    for k in range(M):
        ut = np.asarray(resC[k]["users_out"]).T    # [NPU, 64] sorted order
        vu = selU_sh[k] >= 0
        users_tab[selU_sh[k][vu]] = ut[spos3u[k][np.nonzero(vu)[0]]]
        gt = np.asarray(resC[k]["groups_out"]).T
        vg = selG_sh[k] >= 0
        groups_tab[selG_sh[k][vg]] = gt[spos3g[k][np.nonzero(vg)[0]]]

    return (users_tab[user_inputs], groups_tab[pos_groups],
            groups_tab[neg_groups],
            user_emb[user_inputs], group_emb[pos_groups],
            group_emb[neg_groups])
